# revision 13
# baseline (speedup 1.0000x reference)
"""Multi-head attention Trainium2 kernel (8 NeuronCores).

Problem: x[2,2048,1024] -> MHA(16 heads, d=64) -> out[2,2048,1024], fp32.

Sharding: 2-way data parallel on batch x 4-way tensor parallel on heads.
Core c handles batch c//4 and heads 4*(c%4) .. 4*(c%4)+3 (a 256-wide slice
of the Wq/Wk/Wv columns and Wo rows). Each core returns a partial output
[2048,1024]; the host sums the 4 TP partials per batch and adds the bias
terms (bo, and bv@Wo which is separable because softmax rows sum to 1;
bk drops out of softmax entirely since (q+bq)@bk is constant along keys).

On-core dataflow (all matmuls in fp32r: e8m11 operands, fp32 accumulate):
  xt = x[b].T (host-transposed)      [1024, 2048]
  Q^T = Wq_g^T stationary over xt    [256, 2048]  (+bq, d on partitions)
  K^T likewise (no bias), V natural  [2048, 256]  via xt-stationary matmuls
  S^T[k,q] = K^T(d,k).T @ Q^T(d,q)   2 heads row-packed (d=64 each)
  P = exp(S^T / 32)                  ScalarE, scale fused
  O'^T[d+1,q] = [V|ones].T @ P       ones column gives softmax denominators
  O^T = O'[0:64] * (1/denom) bcast   DMA partition-broadcast + DVE multiply
  out = O^T.T @ Wo_g                 [2048, 1024] partial, DMA'd out
"""

import numpy as np

B = 2
N = 2048
E = 1024
HEADS = 16
D = 64
P = 128
NCORES = 8
GROUPS = 4            # TP groups
DG = E // GROUPS      # 256 cols per core
ECH = E // P          # 8 contraction chunks
NCH = N // P          # 16 sequence chunks
QS = 1024             # q span for softmax tiles
QB = 512              # matmul moving free dim

_CACHE = {}


def _round_f32r(x: np.ndarray) -> np.ndarray:
    """Round fp32 to fp32r (e8m11): RNE on the low 12 mantissa bits."""
    u = np.ascontiguousarray(x, dtype=np.float32).view(np.uint32)
    lower = u & np.uint32(0xFFF)
    base = u & np.uint32(0xFFFFF000)
    up = (lower > np.uint32(1 << 11)) | (
        (lower == np.uint32(1 << 11)) & (((base >> np.uint32(12)) & np.uint32(1)) == 1)
    )
    return (base + np.where(up, np.uint32(1 << 12), np.uint32(0))).view(np.float32)


def _patch_ldw_opt():
    """Enable walrus LDWEIGHTS scheduling opt (pull-ahead); the concourse
    default pins --enable-ldw-opt=false."""
    from concourse import bass_utils
    if getattr(bass_utils, "_ldw_patched", False):
        return
    orig = bass_utils.bir_verify_and_optimise

    def patched(tmpdir, inp="bir.json", outp="file.neff", arch=None, *, dve_root=None):
        import concourse.bass_utils as bu
        real_run = bu.run_command

        def run_hook(cmd, **kw):
            cmd = [c.replace("--enable-ldw-opt=false", "--enable-ldw-opt=true")
                   if isinstance(c, str) else c for c in cmd]
            return real_run(cmd, **kw)

        bu.run_command = run_hook
        try:
            return orig(tmpdir, inp, outp, arch, dve_root=dve_root)
        finally:
            bu.run_command = real_run

    bass_utils.bir_verify_and_optimise = patched
    import concourse.bass_utils
    concourse.bass_utils._ldw_patched = True


def _build():
    import sys
    if "/opt/trn_rl_repo" not in sys.path:
        sys.path.insert(0, "/opt/trn_rl_repo")
    import concourse.tile as tile
    from concourse import bacc, mybir
    from concourse.bass import ts

    F32 = mybir.dt.float32
    F32R = mybir.dt.float32r
    BF16 = mybir.dt.bfloat16
    Exp = mybir.ActivationFunctionType.Exp

    nc = bacc.Bacc("TRN2", target_bir_lowering=False, debug=False, num_devices=NCORES)

    xt = nc.dram_tensor("xt", [E, N], F32R, kind="ExternalInput").ap()
    wq = nc.dram_tensor("wq", [E, DG], F32R, kind="ExternalInput").ap()
    wk = nc.dram_tensor("wk", [E, DG], F32R, kind="ExternalInput").ap()
    wv = nc.dram_tensor("wv", [E, DG], F32R, kind="ExternalInput").ap()
    wo = nc.dram_tensor("wo", [DG, E], F32R, kind="ExternalInput").ap()
    bq2 = nc.dram_tensor("bq2", [P, 2], F32, kind="ExternalInput").ap()
    out = nc.dram_tensor("out", [N, E], F32, kind="ExternalOutput").ap()

    with tile.TileContext(nc) as tc:
        with tc.tile_pool(name="persist", bufs=1) as pers, \
             tc.tile_pool(name="pexp", bufs=6) as pexp_pool, \
             tc.tile_pool(name="small", bufs=2) as small, \
             tc.tile_pool(name="ostage", bufs=4) as ostage, \
             tc.tile_pool(name="ppmain", bufs=1, space="PSUM") as ppm, \
             tc.tile_pool(name="ppoacc", bufs=1, space="PSUM") as ppo:
            wq_sb = pers.tile([P, ECH, DG], F32R, tag="wq")
            wk_sb = pers.tile([P, ECH, DG], F32R, tag="wk")
            wv_sb = pers.tile([P, ECH, DG], F32R, tag="wv")
            wo_sb = pers.tile([P, 2, E], F32R, tag="wo")
            bq_sb = pers.tile([P, 2], F32, tag="bq")
            qT_p = [pers.tile([P, N], BF16, tag=f"qT{i}", name=f"qT{i}") for i in range(2)]
            kT_p = [pers.tile([P, N], BF16, tag=f"kT{i}", name=f"kT{i}") for i in range(2)]
            v_sb = pers.tile([P, NCH, GROUPS, 66], BF16, tag="v")
            oT_p = [pers.tile([P, N], F32R, tag=f"oT{i}", name=f"oT{i}") for i in range(2)]

            def emit_qk(pair, xt_sb):
                # K first (attention needs all of K^T; Q only per q-span)
                for w_sb, dst, bias in ((wk_sb, kT_p[pair], False), (wq_sb, qT_p[pair], True)):
                    for qb in range(N // QB):
                        ps = ppm.tile([P, QS], F32, tag="A" if qb % 2 == 0 else "B",
                                      name=f"qkps{pair}{qb}")
                        psl = ps[:, :QB]
                        for ec in range(ECH):
                            nc.tensor.matmul(
                                psl,
                                w_sb[:, ec, ts(pair, P)],
                                xt_sb[:, ec, ts(qb, QB)],
                                start=(ec == 0), stop=(ec == ECH - 1),
                            )
                        if bias:
                            nc.vector.tensor_add(
                                dst[:, ts(qb, QB)], psl,
                                bq_sb[:, pair, None].to_broadcast((P, QB)),
                            )
                        else:
                            nc.vector.tensor_copy(dst[:, ts(qb, QB)], psl)

            def emit_v(xt_sb):
                for ncx in range(NCH):
                    ps = ppm.tile([P, QS], F32, tag="A" if ncx % 2 == 0 else "B",
                                  name=f"vps{ncx}")
                    psl = ps[:, :DG]
                    for ec in range(ECH):
                        nc.tensor.matmul(
                            psl,
                            xt_sb[:, ec, ts(ncx, P)],
                            wv_sb[:, ec, :],
                            start=(ec == 0), stop=(ec == ECH - 1),
                        )
                    nc.vector.tensor_copy(
                        v_sb[:, ncx, :, 0:64],
                        psl.rearrange("p (h d) -> p h d", d=D),
                    )

            def emit_attn(pair, qs):
                oaccs = [ppo.tile([65, QS], F32, tag=f"O{h}", name=f"oacc{h}")
                         for h in range(2)]
                for kc in range(NCH):
                    pss = [ppm.tile([P, QS], F32, tag="AB"[h], name=f"spsum{h}")
                           for h in range(2)]
                    for qb in range(QS // QB):
                        for h in range(2):
                            psl = slice(D * h, D * h + D)
                            nc.tensor.matmul(
                                pss[h][:, ts(qb, QB)],
                                kT_p[pair][psl, ts(kc, P)],
                                qT_p[pair][psl, qs * QS + qb * QB:qs * QS + (qb + 1) * QB],
                                start=True, stop=True,
                            )
                    for h in range(2):
                        pe = pexp_pool.tile([P, QS], BF16, tag="pexp", name="pexp")
                        nc.scalar.activation(pe, pss[h], Exp, scale=1.0 / 32.0)
                        hh = 2 * pair + h
                        for qb in range(QS // QB):
                            nc.tensor.matmul(
                                oaccs[h][:, ts(qb, QB)],
                                v_sb[:, kc, hh, 0:65],
                                pe[:, ts(qb, QB)],
                                start=(kc == 0), stop=(kc == NCH - 1),
                            )
                for h in range(2):
                    psl = slice(D * h, D * h + D)
                    osp = small.tile([65, QS], F32, tag="osp", name="osp")
                    nc.vector.tensor_copy(osp, oaccs[h])
                    rvec = small.tile([1, QS], F32, tag="rvec", name="rvec")
                    nc.vector.reciprocal(rvec, osp[64:65, :])
                    rbc = small.tile([P, QS], F32, tag="rbc", name="rbc")
                    nc.gpsimd.partition_broadcast(rbc, rvec)
                    nc.vector.tensor_mul(
                        oT_p[pair][psl, ts(qs, QS)],
                        osp[0:64, :],
                        rbc[0:64, :],
                    )

            def emit_wo(nc_lo, nc_hi):
                for ncx in range(nc_lo, nc_hi):
                    for fb in range(E // QB):
                        ps = ppm.tile([P, QS], F32,
                                      tag="A" if (ncx * 2 + fb) % 2 == 0 else "B",
                                      name=f"wops{ncx}{fb}")
                        psl = ps[:, :QB]
                        for dc in range(2):
                            nc.tensor.matmul(
                                psl,
                                oT_p[dc][:, ts(ncx, P)],
                                wo_sb[:, dc, ts(fb, QB)],
                                start=(dc == 0), stop=(dc == 1),
                            )
                        ot = ostage.tile([P, QB], F32, tag="ot", name="ot")
                        nc.vector.tensor_copy(ot, psl)
                        nc.sync.dma_start(out[ts(ncx, P), ts(fb, QB)], ot)

            with tc.tile_pool(name="xtp", bufs=1) as xtp:
                xt_sb = xtp.tile([P, ECH, N], F32R, tag="xt")
                xt_r = xt.rearrange("(c p) n -> p c n", p=P)
                for ncx in range(NCH):
                    nc.sync.dma_start(xt_sb[:, :, ts(ncx, P)], xt_r[:, :, ts(ncx, P)])
                nc.sync.dma_start(wv_sb, wv.rearrange("(c p) d -> p c d", p=P))
                nc.sync.dma_start(wk_sb, wk.rearrange("(c p) d -> p c d", p=P))
                nc.sync.dma_start(wq_sb, wq.rearrange("(c p) d -> p c d", p=P))
                nc.sync.dma_start(wo_sb, wo.rearrange("(c p) f -> p c f", p=P))
                nc.sync.dma_start(bq_sb, bq2)
                ones_f32 = pers.tile([P, 1], F32, tag="ones")
                nc.vector.memset(ones_f32, 1.0)
                nc.vector.tensor_copy(
                    v_sb[:, :, :, 64:65],
                    ones_f32[:, 0, None, None, None].to_broadcast((P, NCH, GROUPS, 1)),
                )
                emit_v(xt_sb)
                emit_qk(0, xt_sb)
                emit_attn(0, 0)
                emit_attn(0, 1)
                emit_qk(1, xt_sb)
            emit_attn(1, 0)
            emit_wo(0, NCH // 2)
            emit_attn(1, 1)
            emit_wo(NCH // 2, NCH)

    nc.compile()
    return nc


def _get_nc():
    if "nc" not in _CACHE:
        _CACHE["nc"] = _build()
    return _CACHE["nc"]


def kernel(x, Wq, bq, Wk, bk, Wv, bv, Wo, bo, **run_kwargs):
    import sys
    if "/opt/trn_rl_repo" not in sys.path:
        sys.path.insert(0, "/opt/trn_rl_repo")
    from concourse.bass_utils import run_bass_kernel_spmd

    x = np.asarray(x, dtype=np.float32)
    Wq = np.asarray(Wq, dtype=np.float32)
    Wk = np.asarray(Wk, dtype=np.float32)
    Wv = np.asarray(Wv, dtype=np.float32)
    Wo = np.asarray(Wo, dtype=np.float32)
    bq = np.asarray(bq, dtype=np.float32)
    bv = np.asarray(bv, dtype=np.float32)
    bo = np.asarray(bo, dtype=np.float32)

    nc = _get_nc()

    in_maps = []
    xts = [_round_f32r(np.ascontiguousarray(x[b].T)) for b in range(B)]
    for c in range(NCORES):
        b, g = divmod(c, GROUPS)
        cols = slice(g * DG, (g + 1) * DG)
        in_maps.append({
            "xt": xts[b],
            "wq": _round_f32r(Wq[:, cols]),
            "wk": _round_f32r(Wk[:, cols]),
            "wv": _round_f32r(Wv[:, cols]),
            "wo": _round_f32r(Wo[cols, :]),
            "bq2": np.ascontiguousarray(bq[cols].reshape(2, P).T),
        })

    res = run_bass_kernel_spmd(nc, in_maps, core_ids=list(range(NCORES)), **run_kwargs)
    if run_kwargs:
        _CACHE["last_results"] = res

    # gather: sum TP partials per batch, add separable bias terms
    bias_vec = bv @ Wo + bo  # softmax rows sum to 1 => bv contributes bv@Wo
    full = np.empty((B, N, E), dtype=np.float32)
    for b in range(B):
        acc = res.results[b * GROUPS]["out"].astype(np.float32).copy()
        for g in range(1, GROUPS):
            acc += res.results[b * GROUPS + g]["out"]
        full[b] = acc + bias_vec[None, :]
    return full
